# revision 22
# baseline (speedup 1.0000x reference)
"""Trainium2 Bass kernel: causal GQA attention (prefill), 8-core tensor-parallel.

Problem: q [4096, 16*128], k/v [4096, 4*128], f32. 16 query heads, 4 kv heads,
head_dim 128, causal softmax(q k^T / sqrt(d)) v.

Sharding: head-parallel across 8 NeuronCores. Core c owns query heads
{2c, 2c+1}, which both belong to kv head c//2. Each core runs full causal
attention over its 2 heads; no cross-core communication.

Per-core kernel (N=4096 tokens, 32 tiles of 128):
  - Inputs DMA'd in large chunks into f32 staging, PE-transposed (dedicated
    1-bank PSUM tag) into qTall [d=128, tile, head, 128] / kT [d=128, 4096]
    bf16; the PSUM->SBUF copy is fused with the f32->bf16 cast. v is cast to
    bf16 with a ones-column appended (vones) so the PV matmul's output column
    128 accumulates the softmax denominator.
  - Both query heads are fused into one score row: for query tile i,
    S^T[m, 256] = kT_j.T @ qTall[:,i] covers head0|head1 side by side. Blocks
    for j=0..i are packed 4-deep into [128, <=1024] PSUM strips, exp'd by one
    wide ScalarE activation (scale=1/sqrt(d) folded in) -> pT bf16, which is
    directly the stationary operand for PV: acc[q,129] += pT_j.T @ [v_j|1].
  - The per-group accumulator pair lives in ONE PSUM bank. matmul start=True
    clears has_written for the whole bank, so instead a single dummy matmul
    (zeros stationary) zero-fills both accumulators once and every PV matmul
    accumulates with start=False. The dummy is emitted lazily before the
    first PV so the next group's QK stream isn't queued behind the previous
    group's normalize.
  - Causal: only blocks j<=i computed; the diagonal block gets [mask|mask]
    written into PSUM by one start=True PE matmul (maskT.T @ [I|I]), scores
    accumulate on top -- no VectorE hop in the QK->exp chain.
  - Normalize: out[q, d] = acc[:, :128] * reciprocal(acc[:, 128]) on VectorE.
  - Transpose/cast prep work is interleaved between score strips two groups
    ahead of use, so TensorE never drains and HAM stays warm.
"""

import sys

for _p in ("/opt/trn_rl_repo",):
    if _p not in sys.path:
        sys.path.insert(0, _p)

import numpy as np

import concourse.bacc as bacc
import concourse.mybir as mybir
import concourse.tile as tile
from concourse.bass_utils import run_bass_kernel_spmd
from concourse.masks import make_identity

F32 = mybir.dt.float32
BF16 = mybir.dt.bfloat16

N = 4096
D = 128
H_PER_CORE = 2
NCORES = 8
NT = N // 128          # 32 token tiles; one fused (2-head) group per tile
SCALE = float(1.0 / np.sqrt(np.float32(D)))
MASK_VAL = -1e9
DMA_CHUNKS = (2, 2, 4, 4, 4, 8, 8)   # token tiles per input DMA instruction


def _build():
    nc = bacc.Bacc(
        "TRN2",
        target_bir_lowering=False,
        debug=False,
        enable_asserts=False,
        num_devices=NCORES,
    )
    q_d = nc.dram_tensor("q", [N, H_PER_CORE * D], F32, kind="ExternalInput").ap()
    k_d = nc.dram_tensor("k", [N, D], F32, kind="ExternalInput").ap()
    v_d = nc.dram_tensor("v", [N, D], F32, kind="ExternalInput").ap()
    o_d = nc.dram_tensor("out", [N, H_PER_CORE * D], F32, kind="ExternalOutput").ap()

    with tile.TileContext(nc) as tc:
        with (
            tc.tile_pool(name="consts", bufs=1) as consts,
            tc.tile_pool(name="big", bufs=1) as big,
            tc.tile_pool(name="cstage", bufs=8) as cstage,
            tc.tile_pool(name="pstage", bufs=6) as pstage,
            tc.tile_pool(name="outp", bufs=4) as outp,
            tc.tile_pool(name="rpool", bufs=4) as rpool,
            tc.tile_pool(name="pst", bufs=3, space="PSUM") as psum_st,
            tc.tile_pool(name="ptp", bufs=1, space="PSUM") as psum_tp,
            tc.tile_pool(name="pacc", bufs=1, space="PSUM") as psum_acc,
        ):
            identity = consts.tile([128, 128], BF16)
            make_identity(nc, identity)

            # diag mask stored TRANSPOSED (maskT[q, m] = 0 if m <= q else
            # MASK_VAL) so maskT.T @ [I|I] writes mask[m, q] for both heads.
            maskT = consts.tile([128, 128], BF16)
            nc.gpsimd.memset(maskT, 0.0)
            nc.gpsimd.affine_select(
                out=maskT,
                in_=maskT,
                compare_op=mybir.AluOpType.is_ge,
                fill=MASK_VAL,
                base=0,
                pattern=[[-1, 128]],
                channel_multiplier=1,
            )
            zeros_bf = consts.tile([128, 128], BF16)
            nc.vector.memset(zeros_bf, 0.0)
            ipair = consts.tile([128, 258], BF16)
            nc.vector.memset(ipair, 0.0)
            make_identity(nc, ipair[:, 0:128], nomemset=True)
            make_identity(nc, ipair[:, 128:256], nomemset=True)

            # f32 staging ([p, tile, col], p = token % 128)
            qst = big.tile([128, NT, H_PER_CORE * D], F32, tag="qst")
            kst = big.tile([128, NT, D], F32, tag="kst")
            vst = big.tile([128, NT, D], F32, tag="vst")

            qTall = big.tile([128, NT, H_PER_CORE, D], BF16, tag="qTall")
            kT = big.tile([128, N], BF16, tag="kT")
            vones = big.tile([128, NT, 129], BF16, tag="vones")

            # ---- chunked input DMAs. k is needed breadth-first (group i
            # reads kT[0..i]), so all k chunks are issued right after the
            # first small q/k/v chunks; q and v follow at demand rate.
            def dma_in(dst, src, t0, t1):
                nc.sync.dma_start(
                    out=dst[:, t0:t1, :],
                    in_=src[t0 * 128 : t1 * 128, :].rearrange(
                        "(t p) c -> p t c", p=128
                    ),
                )

            dma_in(qst, q_d, 0, 2)
            dma_in(kst, k_d, 0, 2)
            dma_in(vst, v_d, 0, 2)
            for t0, t1 in ((2, 8), (8, 16), (16, 24), (24, 32)):
                dma_in(kst, k_d, t0, t1)
            for t0, t1 in ((2, 8), (8, 16), (16, 24), (24, 32)):
                dma_in(qst, q_d, t0, t1)
                dma_in(vst, v_d, t0, t1)

            def do_prep(unit):
                kind = unit[0]
                t = unit[-1]
                # early tiles borrow the 3-slot "st" tag so the transpose
                # pipeline isn't serialized through the single "tp" bank
                # during the ramp, when score strips are still sparse
                if t <= 15:
                    tpool, ttag = psum_st, "st"
                else:
                    tpool, ttag = psum_tp, "tp"
                if kind == "k":
                    cb = cstage.tile([128, 128], BF16, tag="cst", name="cbk")
                    nc.gpsimd.tensor_copy(cb, kst[:, t, :])
                    tp = tpool.tile([128, 128], BF16, tag=ttag, name="tpk")
                    nc.tensor.transpose(tp, cb, identity)
                    nc.vector.tensor_copy(kT[:, t * 128 : (t + 1) * 128], tp)
                elif kind == "q":
                    _, h, t = unit
                    cb = cstage.tile([128, 128], BF16, tag="cst", name="cbq")
                    nc.gpsimd.tensor_copy(cb, qst[:, t, h * D : (h + 1) * D])
                    tp = tpool.tile([128, 128], BF16, tag=ttag, name="tpq")
                    nc.tensor.transpose(tp, cb, identity)
                    nc.vector.tensor_copy(qTall[:, t, h, :], tp)
                else:  # v cast, 4-tile granularity
                    t = unit[1]
                    nc.gpsimd.tensor_copy(
                        vones[:, t : t + 4, 0:128], vst[:, t : t + 4, :]
                    )
                    nc.gpsimd.memset(vones[:, t : t + 4, 128:129], 1.0)

            # upfront prep: tiles 0..1 (groups 0, 1)
            for t in range(2):
                do_prep(("k", t))
                do_prep(("q", 0, t))
                do_prep(("q", 1, t))
            do_prep(("v", 0))

            # ---- flat strip stream, software-pipelined one strip ahead ----
            # Strip n+1's QK matmuls + exp are emitted BEFORE strip n's PV
            # matmuls, so at group boundaries ScalarE always has the next
            # score strip ready and never drains.
            strips_flat = []
            for i in range(NT):
                blocks = list(range(i + 1))
                chunks = [blocks[x : x + 4] for x in range(0, len(blocks), 4)]
                for si, strip in enumerate(chunks):
                    strips_flat.append((i, strip, si == len(chunks) - 1))

            group_state = {}  # i -> (acc2, [accs], dummy_emitted)

            def emit_qk_exp(n):
                i, strip, _last = strips_flat[n]
                st2 = psum_st.tile([128, 1024], F32, tag="st", name="st2")
                pt2 = pstage.tile([128, 1024], BF16, tag="pt", name="pt2")
                for bi, j in enumerate(strip):
                    so = bi * 256
                    diag = j == i
                    if diag:
                        nc.tensor.matmul(
                            st2[:, so : so + 256],
                            lhsT=maskT,
                            rhs=ipair[:, 0:256],
                            start=True,
                            stop=True,
                        )
                    nc.tensor.matmul(
                        st2[:, so : so + 256],
                        lhsT=kT[:, j * 128 : (j + 1) * 128],
                        rhs=qTall[:, i, :, :],
                        start=not diag,
                        stop=True,
                    )
                so_end = len(strip) * 256
                nc.scalar.activation(
                    out=pt2[:, 0:so_end],
                    in_=st2[:, 0:so_end],
                    func=mybir.ActivationFunctionType.Exp,
                    scale=SCALE,
                )
                return pt2

            def emit_pv(n, pt2):
                i, strip, last = strips_flat[n]
                if i not in group_state:
                    acc2 = psum_acc.tile(
                        [128, H_PER_CORE, 129], F32, tag="acc", name="acc2"
                    )
                    nc.tensor.matmul(
                        acc2.rearrange("p a c -> p (a c)"),
                        lhsT=zeros_bf,
                        rhs=ipair[:, 0 : H_PER_CORE * 129],
                        start=True,
                        stop=True,
                    )
                    group_state[i] = acc2
                acc2 = group_state[i]
                for bi, j in enumerate(strip):
                    so = bi * 256
                    for hh in range(H_PER_CORE):
                        nc.tensor.matmul(
                            acc2[:, hh, :],
                            lhsT=pt2[:, so + hh * 128 : so + (hh + 1) * 128],
                            rhs=vones[:, j, :],
                            start=False,
                            stop=(j == i),
                        )
                if last:
                    for hh in range(H_PER_CORE):
                        rec = rpool.tile([128, 1], F32, tag="rec", name="rec")
                        nc.vector.reciprocal(rec, acc2[:, hh, 128:129])
                        ot = outp.tile([128, 128], F32, tag="ot", name="ot")
                        nc.vector.tensor_scalar_mul(ot, acc2[:, hh, 0:128], rec)
                        nc.sync.dma_start(
                            out=o_d[i * 128 : (i + 1) * 128, hh * D : (hh + 1) * D],
                            in_=ot,
                        )

            # prep schedule: group i's tiles must be ready 2 groups early;
            # attach prep units to the first strip of group i
            prep_at = {}
            for i in range(NT):
                tn = i + 2
                if tn < NT:
                    units = [("k", tn), ("q", 0, tn), ("q", 1, tn)]
                    if tn % 4 == 0:
                        units.append(("v", tn))
                    prep_at[i] = units

            pending_preps = []
            cur_pt = emit_qk_exp(0)
            for n in range(len(strips_flat)):
                i = strips_flat[n][0]
                if strips_flat[n][1][0] == 0:  # first strip of group i
                    pending_preps.extend(prep_at.get(i, []))
                nxt_pt = emit_qk_exp(n + 1) if n + 1 < len(strips_flat) else None
                emit_pv(n, cur_pt)
                for _ in range(5 if n < 40 else 3):
                    if pending_preps:
                        do_prep(pending_preps.pop(0))
                cur_pt = nxt_pt
            for p in pending_preps:
                do_prep(p)

    nc.compile()
    return nc


_NC = None


def _get_nc():
    global _NC
    if _NC is None:
        _NC = _build()
    return _NC


def _shard(q, k, v):
    in_maps = []
    for c in range(NCORES):
        g = c // 2
        in_maps.append(
            {
                "q": np.ascontiguousarray(
                    q[:, c * H_PER_CORE * D : (c + 1) * H_PER_CORE * D],
                    dtype=np.float32,
                ),
                "k": np.ascontiguousarray(k[:, g * D : (g + 1) * D], dtype=np.float32),
                "v": np.ascontiguousarray(v[:, g * D : (g + 1) * D], dtype=np.float32),
            }
        )
    return in_maps


def _run(q, k, v, trace=False):
    nc = _get_nc()
    res = run_bass_kernel_spmd(
        nc, _shard(q, k, v), core_ids=list(range(NCORES)), trace=trace
    )
    out = np.concatenate(
        [np.asarray(res.results[c]["out"]) for c in range(NCORES)], axis=1
    )
    return out.astype(np.float32, copy=False), res


def kernel(q, k, v):
    out, _ = _run(np.asarray(q), np.asarray(k), np.asarray(v), trace=False)
    return out


# revision 23
# speedup vs baseline: 1.0230x; 1.0230x over previous
"""Trainium2 Bass kernel: causal GQA attention (prefill), 8-core tensor-parallel.

Problem: q [4096, 16*128], k/v [4096, 4*128], f32. 16 query heads, 4 kv heads,
head_dim 128, causal softmax(q k^T / sqrt(d)) v.

Sharding: head-parallel across 8 NeuronCores. Core c owns query heads
{2c, 2c+1}, which both belong to kv head c//2. Each core runs full causal
attention over its 2 heads; no cross-core communication.

Per-core kernel (N=4096 tokens, 32 tiles of 128):
  - Inputs DMA'd in large chunks into f32 staging, PE-transposed (dedicated
    1-bank PSUM tag) into qTall [d=128, tile, head, 128] / kT [d=128, 4096]
    bf16; the PSUM->SBUF copy is fused with the f32->bf16 cast. v is cast to
    bf16 with a ones-column appended (vones) so the PV matmul's output column
    128 accumulates the softmax denominator.
  - Both query heads are fused into one score row: for query tile i,
    S^T[m, 256] = kT_j.T @ qTall[:,i] covers head0|head1 side by side. Blocks
    for j=0..i are packed 4-deep into [128, <=1024] PSUM strips, exp'd by one
    wide ScalarE activation (scale=1/sqrt(d) folded in) -> pT bf16, which is
    directly the stationary operand for PV: acc[q,129] += pT_j.T @ [v_j|1].
  - The per-group accumulator pair lives in ONE PSUM bank. matmul start=True
    clears has_written for the whole bank, so instead a single dummy matmul
    (zeros stationary) zero-fills both accumulators once and every PV matmul
    accumulates with start=False. The dummy is emitted lazily before the
    first PV so the next group's QK stream isn't queued behind the previous
    group's normalize.
  - Causal: only blocks j<=i computed; the diagonal block gets [mask|mask]
    written into PSUM by one start=True PE matmul (maskT.T @ [I|I]), scores
    accumulate on top -- no VectorE hop in the QK->exp chain.
  - Normalize: out[q, d] = acc[:, :128] * reciprocal(acc[:, 128]) on VectorE.
  - Transpose/cast prep work is interleaved between score strips two groups
    ahead of use, so TensorE never drains and HAM stays warm.
"""

import sys

for _p in ("/opt/trn_rl_repo",):
    if _p not in sys.path:
        sys.path.insert(0, _p)

import numpy as np

import concourse.bacc as bacc
import concourse.mybir as mybir
import concourse.tile as tile
from concourse.bass_utils import run_bass_kernel_spmd
from concourse.masks import make_identity

F32 = mybir.dt.float32
BF16 = mybir.dt.bfloat16

N = 4096
D = 128
H_PER_CORE = 2
NCORES = 8
NT = N // 128          # 32 token tiles; one fused (2-head) group per tile
SCALE = float(1.0 / np.sqrt(np.float32(D)))
MASK_VAL = -1e9
DMA_CHUNKS = (2, 2, 4, 4, 4, 8, 8)   # token tiles per input DMA instruction


def _build():
    nc = bacc.Bacc(
        "TRN2",
        target_bir_lowering=False,
        debug=False,
        enable_asserts=False,
        num_devices=NCORES,
    )
    q_d = nc.dram_tensor("q", [N, H_PER_CORE * D], F32, kind="ExternalInput").ap()
    k_d = nc.dram_tensor("k", [N, D], F32, kind="ExternalInput").ap()
    v_d = nc.dram_tensor("v", [N, D], F32, kind="ExternalInput").ap()
    o_d = nc.dram_tensor("out", [N, H_PER_CORE * D], F32, kind="ExternalOutput").ap()

    with tile.TileContext(nc) as tc:
        with (
            tc.tile_pool(name="consts", bufs=1) as consts,
            tc.tile_pool(name="big", bufs=1) as big,
            tc.tile_pool(name="cstage", bufs=8) as cstage,
            tc.tile_pool(name="pstage", bufs=6) as pstage,
            tc.tile_pool(name="outp", bufs=4) as outp,
            tc.tile_pool(name="rpool", bufs=4) as rpool,
            tc.tile_pool(name="pst", bufs=3, space="PSUM") as psum_st,
            tc.tile_pool(name="ptp", bufs=1, space="PSUM") as psum_tp,
            tc.tile_pool(name="pacc", bufs=1, space="PSUM") as psum_acc,
        ):
            identity = consts.tile([128, 128], BF16)
            make_identity(nc, identity)

            # diag mask stored TRANSPOSED (maskT[q, m] = 0 if m <= q else
            # MASK_VAL) so maskT.T @ [I|I] writes mask[m, q] for both heads.
            maskT = consts.tile([128, 128], BF16)
            nc.gpsimd.memset(maskT, 0.0)
            nc.gpsimd.affine_select(
                out=maskT,
                in_=maskT,
                compare_op=mybir.AluOpType.is_ge,
                fill=MASK_VAL,
                base=0,
                pattern=[[-1, 128]],
                channel_multiplier=1,
            )
            zeros_bf = consts.tile([128, 128], BF16)
            nc.vector.memset(zeros_bf, 0.0)
            ipair = consts.tile([128, 258], BF16)
            nc.vector.memset(ipair, 0.0)
            make_identity(nc, ipair[:, 0:128], nomemset=True)
            make_identity(nc, ipair[:, 128:256], nomemset=True)

            # f32 staging ([p, tile, col], p = token % 128)
            qst = big.tile([128, NT, H_PER_CORE * D], F32, tag="qst")
            kst = big.tile([128, NT, D], F32, tag="kst")
            vst = big.tile([128, NT, D], F32, tag="vst")

            qTall = big.tile([128, NT, H_PER_CORE, D], BF16, tag="qTall")
            kT = big.tile([128, N], BF16, tag="kT")
            vones = big.tile([128, NT, 129], BF16, tag="vones")

            # ---- chunked input DMAs. k is needed breadth-first (group i
            # reads kT[0..i]), so all k chunks are issued right after the
            # first small q/k/v chunks; q and v follow at demand rate.
            def dma_in(dst, src, t0, t1):
                nc.sync.dma_start(
                    out=dst[:, t0:t1, :],
                    in_=src[t0 * 128 : t1 * 128, :].rearrange(
                        "(t p) c -> p t c", p=128
                    ),
                )

            dma_in(qst, q_d, 0, 2)
            dma_in(kst, k_d, 0, 2)
            dma_in(vst, v_d, 0, 2)
            for t0, t1 in ((2, 8), (8, 16), (16, 24), (24, 32)):
                dma_in(kst, k_d, t0, t1)
            for t0, t1 in ((2, 8), (8, 16), (16, 24), (24, 32)):
                dma_in(qst, q_d, t0, t1)
                dma_in(vst, v_d, t0, t1)

            def do_prep(unit):
                kind = unit[0]
                t = unit[-1]
                # early tiles borrow the 3-slot "st" tag so the transpose
                # pipeline isn't serialized through the single "tp" bank
                # during the ramp, when score strips are still sparse
                if t <= 15:
                    tpool, ttag = psum_st, "st"
                else:
                    tpool, ttag = psum_tp, "tp"
                if kind == "k":
                    cb = cstage.tile([128, 128], BF16, tag="cst", name="cbk")
                    nc.vector.tensor_copy(cb, kst[:, t, :])
                    tp = tpool.tile([128, 128], BF16, tag=ttag, name="tpk")
                    nc.tensor.transpose(tp, cb, identity)
                    nc.vector.tensor_copy(kT[:, t * 128 : (t + 1) * 128], tp)
                elif kind == "q":
                    _, h, t = unit
                    cb = cstage.tile([128, 128], BF16, tag="cst", name="cbq")
                    nc.vector.tensor_copy(cb, qst[:, t, h * D : (h + 1) * D])
                    tp = tpool.tile([128, 128], BF16, tag=ttag, name="tpq")
                    nc.tensor.transpose(tp, cb, identity)
                    nc.vector.tensor_copy(qTall[:, t, h, :], tp)
                else:  # v cast, 4-tile granularity
                    t = unit[1]
                    nc.vector.tensor_copy(
                        vones[:, t : t + 4, 0:128], vst[:, t : t + 4, :]
                    )
                    nc.vector.memset(vones[:, t : t + 4, 128:129], 1.0)

            # upfront prep: tiles 0..1 (groups 0, 1)
            for t in range(2):
                do_prep(("k", t))
                do_prep(("q", 0, t))
                do_prep(("q", 1, t))
            do_prep(("v", 0))

            # ---- flat strip stream, software-pipelined one strip ahead ----
            # Strip n+1's QK matmuls + exp are emitted BEFORE strip n's PV
            # matmuls, so at group boundaries ScalarE always has the next
            # score strip ready and never drains.
            strips_flat = []
            for i in range(NT):
                blocks = list(range(i + 1))
                chunks = [blocks[x : x + 4] for x in range(0, len(blocks), 4)]
                for si, strip in enumerate(chunks):
                    strips_flat.append((i, strip, si == len(chunks) - 1))

            group_state = {}  # i -> (acc2, [accs], dummy_emitted)

            def emit_qk_exp(n):
                i, strip, _last = strips_flat[n]
                st2 = psum_st.tile([128, 1024], F32, tag="st", name="st2")
                pt2 = pstage.tile([128, 1024], BF16, tag="pt", name="pt2")
                for bi, j in enumerate(strip):
                    so = bi * 256
                    diag = j == i
                    if diag:
                        nc.tensor.matmul(
                            st2[:, so : so + 256],
                            lhsT=maskT,
                            rhs=ipair[:, 0:256],
                            start=True,
                            stop=True,
                        )
                    nc.tensor.matmul(
                        st2[:, so : so + 256],
                        lhsT=kT[:, j * 128 : (j + 1) * 128],
                        rhs=qTall[:, i, :, :],
                        start=not diag,
                        stop=True,
                    )
                so_end = len(strip) * 256
                nc.scalar.activation(
                    out=pt2[:, 0:so_end],
                    in_=st2[:, 0:so_end],
                    func=mybir.ActivationFunctionType.Exp,
                    scale=SCALE,
                )
                return pt2

            def emit_pv(n, pt2):
                i, strip, last = strips_flat[n]
                if i not in group_state:
                    acc2 = psum_acc.tile(
                        [128, H_PER_CORE, 129], F32, tag="acc", name="acc2"
                    )
                    nc.tensor.matmul(
                        acc2.rearrange("p a c -> p (a c)"),
                        lhsT=zeros_bf,
                        rhs=ipair[:, 0 : H_PER_CORE * 129],
                        start=True,
                        stop=True,
                    )
                    group_state[i] = acc2
                acc2 = group_state[i]
                for bi, j in enumerate(strip):
                    so = bi * 256
                    for hh in range(H_PER_CORE):
                        nc.tensor.matmul(
                            acc2[:, hh, :],
                            lhsT=pt2[:, so + hh * 128 : so + (hh + 1) * 128],
                            rhs=vones[:, j, :],
                            start=False,
                            stop=(j == i),
                        )
                if last:
                    for hh in range(H_PER_CORE):
                        rec = rpool.tile([128, 1], F32, tag="rec", name="rec")
                        nc.vector.reciprocal(rec, acc2[:, hh, 128:129])
                        ot = outp.tile([128, 128], F32, tag="ot", name="ot")
                        nc.vector.tensor_scalar_mul(ot, acc2[:, hh, 0:128], rec)
                        nc.sync.dma_start(
                            out=o_d[i * 128 : (i + 1) * 128, hh * D : (hh + 1) * D],
                            in_=ot,
                        )

            # prep schedule: group i's tiles must be ready 2 groups early;
            # attach prep units to the first strip of group i
            prep_at = {}
            for i in range(NT):
                tn = i + 2
                if tn < NT:
                    units = [("k", tn), ("q", 0, tn), ("q", 1, tn)]
                    if tn % 4 == 0:
                        units.append(("v", tn))
                    prep_at[i] = units

            pending_preps = []
            cur_pt = emit_qk_exp(0)
            for n in range(len(strips_flat)):
                i = strips_flat[n][0]
                if strips_flat[n][1][0] == 0:  # first strip of group i
                    pending_preps.extend(prep_at.get(i, []))
                nxt_pt = emit_qk_exp(n + 1) if n + 1 < len(strips_flat) else None
                emit_pv(n, cur_pt)
                for _ in range(5 if n < 40 else 3):
                    if pending_preps:
                        do_prep(pending_preps.pop(0))
                cur_pt = nxt_pt
            for p in pending_preps:
                do_prep(p)

    nc.compile()
    return nc


_NC = None


def _get_nc():
    global _NC
    if _NC is None:
        _NC = _build()
    return _NC


def _shard(q, k, v):
    in_maps = []
    for c in range(NCORES):
        g = c // 2
        in_maps.append(
            {
                "q": np.ascontiguousarray(
                    q[:, c * H_PER_CORE * D : (c + 1) * H_PER_CORE * D],
                    dtype=np.float32,
                ),
                "k": np.ascontiguousarray(k[:, g * D : (g + 1) * D], dtype=np.float32),
                "v": np.ascontiguousarray(v[:, g * D : (g + 1) * D], dtype=np.float32),
            }
        )
    return in_maps


def _run(q, k, v, trace=False):
    nc = _get_nc()
    res = run_bass_kernel_spmd(
        nc, _shard(q, k, v), core_ids=list(range(NCORES)), trace=trace
    )
    out = np.concatenate(
        [np.asarray(res.results[c]["out"]) for c in range(NCORES)], axis=1
    )
    return out.astype(np.float32, copy=False), res


def kernel(q, k, v):
    out, _ = _run(np.asarray(q), np.asarray(k), np.asarray(v), trace=False)
    return out
